# revision 15
# baseline (speedup 1.0000x reference)
"""Deformable Conv1d (B=4, C=256, L=8192, K=3, DG=4) on 8 Trainium2 cores.

Sharding: core = (sample b = core//2, L-half h = core%2); each core computes
out[b, :, h*4096:(h+1)*4096] from a haloed window of x[b].

Per-core pipeline:
  conv (PE, bf16): offset/mask convs as shifted-rhs matmuls + an iota matmul
    so PSUM holds p = off + l + (k-1) + HALO after the per-row drain bias.
  small chain (DVE, packed [96,512]): t = p mod 1, a0=(1-t)*m, a1=t*m,
    idx = int16(p - t) clamped.
  dma_gather (transpose=True) from per-dgroup transposed pair tables:
    row e = [x[c, e] for c in dgroup] ++ [x[c, e+1] for c]  (256B bf16 rows).
  A-broadcast (PE selector matmul): replicates a0 to partitions 0-63 and a1
    to 64-127, reading the a-plane through a sigma_c-permuted rhs AP.
  modulate (DVE): M = G * A;  S[64*(dk%2)+...] = M[0:64] + M[64:128].
  main matmul (PE): out = W2 @ S + bias with sigma_c-unpermuting rhs AP.
"""
import sys
sys.path.insert(0, '/opt/trn_rl_repo')
from contextlib import ExitStack
import numpy as np
import ml_dtypes

import concourse.bass as bass
import concourse.tile as tile
from concourse import bacc, mybir

dt = mybir.dt
bf16 = ml_dtypes.bfloat16

B, C, L = 4, 256, 8192
N_CORES = 8
LH = L // 2
HALO = 17
W = LH + 2 * HALO          # 4130 window positions
WROWS = 33 * 128           # 4224 padded rows in pair tables
NCHUNK = 8
CH = 512
NBATCH = 4                 # 2-chunk modulate batches
BCH = 2 * CH
AF = mybir.ActivationFunctionType
ALU = mybir.AluOpType


def build_program(n_reps=1):
    nc = bacc.Bacc("TRN2", target_bir_lowering=False, debug=False,
                   enable_asserts=True, num_devices=N_CORES)

    def din(name, shape, dty):
        return nc.dram_tensor(name, shape, dty, kind="ExternalInput").ap()

    xT = din("xT", (4, 128, WROWS), dt.bfloat16)
    xP = din("xP", (2, 128, W), dt.bfloat16)
    wconv = din("wconv", (2, 3, 128, 44), dt.bfloat16)
    iotas = din("iotas", (2, 128, CH), dt.float32)
    boff = din("boff", (12, 1), dt.float32)
    bmask = din("bmask", (12, 1), dt.float32)
    wmain = din("wmain", (6, 2, 128, 128), dt.bfloat16)
    bmain = din("bmain", (2, 128, 1), dt.float32)
    wsel = din("wsel", (12, 24, 128), dt.bfloat16)
    yout = nc.dram_tensor("y", (2, 128, LH), dt.float32, kind="ExternalOutput").ap()

    with ExitStack() as ctx:
        tc = ctx.enter_context(tile.TileContext(nc))
        cpool = ctx.enter_context(tc.tile_pool(name="const", bufs=1))
        ppool = ctx.enter_context(tc.tile_pool(name="planes", bufs=1))
        gpool = ctx.enter_context(tc.tile_pool(name="g", bufs=1))
        mpool = ctx.enter_context(tc.tile_pool(name="mtp", bufs=1))
        apool = ctx.enter_context(tc.tile_pool(name="a", bufs=1))
        spool = ctx.enter_context(tc.tile_pool(name="s", bufs=1))
        opool = ctx.enter_context(tc.tile_pool(name="o", bufs=2))
        psc = ctx.enter_context(tc.tile_pool(name="psc", bufs=2, space="PSUM"))
        psb = ctx.enter_context(tc.tile_pool(name="psb", bufs=2, space="PSUM"))
        psm = ctx.enter_context(tc.tile_pool(name="psm", bufs=2, space="PSUM"))

        t_xT = [cpool.tile([128, WROWS], dt.bfloat16, tag=f"xT{d}", name=f"xT{d}") for d in range(4)]
        for d in range(4):
            nc.sync.dma_start(t_xT[d][:], xT[d])
        t_xP = [cpool.tile([128, W], dt.bfloat16, tag=f"xP{cb}", name=f"xP{cb}") for cb in range(2)]
        for cb in range(2):
            nc.sync.dma_start(t_xP[cb][:], xP[cb])
        t_wconv = [[cpool.tile([128, 44], dt.bfloat16, tag=f"wc{cb}{k}", name=f"wc{cb}{k}")
                    for k in range(3)] for cb in range(2)]
        for cb in range(2):
            for k in range(3):
                nc.sync.dma_start(t_wconv[cb][k][:], wconv[cb, k])
        t_iot = [cpool.tile([128, CH], dt.float32, tag=f"iot{t}", name=f"iot{t}")
                 for t in range(2)]
        for t in range(2):
            nc.sync.dma_start(t_iot[t][:], iotas[t])
        t_boff = cpool.tile([12, 1], dt.float32, name="boff")
        nc.sync.dma_start(t_boff[:], boff[:])
        t_bmask = cpool.tile([12, 1], dt.float32, name="bmask")
        nc.sync.dma_start(t_bmask[:], bmask[:])
        t_wmain = [[cpool.tile([128, 128], dt.bfloat16, tag=f"wm{kb}{mb}", name=f"wm{kb}{mb}")
                    for mb in range(2)] for kb in range(6)]
        for kb in range(6):
            for mb in range(2):
                nc.sync.dma_start(t_wmain[kb][mb][:], wmain[kb, mb])
        t_bmain = [cpool.tile([128, 1], dt.float32, tag=f"bm{mb}", name=f"bm{mb}") for mb in range(2)]
        for mb in range(2):
            nc.sync.dma_start(t_bmain[mb][:], bmain[mb])
        t_wsel = [cpool.tile([24, 128], dt.bfloat16, tag=f"sel{dk}", name=f"sel{dk}") for dk in range(12)]
        for dk in range(12):
            nc.sync.dma_start(t_wsel[dk][:], wsel[dk])

        for _rep in range(n_reps):
            pk = [ppool.tile([128, CH], dt.float32, tag=f"pk{t}", name=f"pk{t}") for t in range(2)]
            mk = [ppool.tile([128, CH], dt.float32, tag=f"mk{t}", name=f"mk{t}") for t in range(2)]
            ttl = [ppool.tile([128, CH], dt.float32, tag=f"ttl{t}", name=f"ttl{t}") for t in range(2)]
            p0f = [ppool.tile([128, CH], dt.float32, tag=f"p0f{t}", name=f"p0f{t}") for t in range(2)]
            i32 = [ppool.tile([128, CH], dt.int32, tag=f"i32{t}", name=f"i32{t}") for t in range(2)]
            a0p = [ppool.tile([128, CH], dt.bfloat16, tag=f"a0p{t}", name=f"a0p{t}") for t in range(2)]
            a1p = [ppool.tile([128, CH], dt.bfloat16, tag=f"a1p{t}", name=f"a1p{t}") for t in range(2)]
            i16p = [ppool.tile([128, CH], dt.int16, tag=f"i16p{t}", name=f"i16p{t}") for t in range(2)]
            aplane = ppool.tile([24, LH], dt.bfloat16, tag="aplane", name="aplane")
            idxt = [ppool.tile([128, LH // 16], dt.int16, tag=f"idx{dk}", name=f"idx{dk}")
                    for dk in range(12)]

            # ---- conv + drains --------------------------------------------
            for c in range(NCHUNK):
                ps = psc.tile([64, CH], dt.float32, tag="convps", name="convps")
                for cb in range(2):
                    for k in range(3):
                        rhs = t_xP[cb][:, c * CH + HALO - 1 + k:
                                       c * CH + HALO - 1 + k + CH]
                        nc.tensor.matmul(ps[0:44, :], t_wconv[cb][k][:], rhs,
                                         start=(cb == 0 and k == 0),
                                         stop=(cb == 1 and k == 2))
                t, rb = c // 4, 32 * (c % 4)
                nc.scalar.activation(pk[t][rb:rb + 12, :], ps[0:12, :], AF.Identity,
                                     bias=t_boff[:], scale=1.0)
                nc.scalar.activation(mk[t][rb:rb + 12, :], ps[32:44, :], AF.Sigmoid,
                                     bias=t_bmask[:], scale=1.0)

            # ---- small chain (packed [96, CH]) ----------------------------
            for t in range(2):
                nc.vector.tensor_add(pk[t][:], pk[t][:], t_iot[t][:])
                nc.vector.tensor_copy(i32[t][:], pk[t][:])          # ~round/trunc
                nc.vector.tensor_copy(p0f[t][:], i32[t][:])
                nc.vector.tensor_sub(ttl[t][:], pk[t][:], p0f[t][:])  # d = p - i
                # floor fixup: if d < 0: i -= 1, d += 1
                nc.vector.tensor_scalar(pk[t][:], ttl[t][:], 0.0, None, ALU.is_lt)
                nc.vector.tensor_sub(p0f[t][:], p0f[t][:], pk[t][:])
                nc.vector.tensor_add(ttl[t][:], ttl[t][:], pk[t][:])  # t in [0,1)
                nc.vector.tensor_mul(ttl[t][:], ttl[t][:], mk[t][:])
                nc.vector.tensor_copy(a1p[t][:], ttl[t][:])
                nc.vector.tensor_sub(a0p[t][:], mk[t][:], ttl[t][:])
                nc.vector.tensor_copy(i16p[t][:], p0f[t][:])
                nc.vector.tensor_scalar_max(i16p[t][:], i16p[t][:], 0)
                nc.vector.tensor_scalar_min(i16p[t][:], i16p[t][:], W - 1)

            # ---- unpack to planes -----------------------------------------
            for c in range(NCHUNK):
                t, rb = c // 4, 32 * (c % 4)
                nc.sync.dma_start(aplane[0:12, c * CH:(c + 1) * CH],
                                  a0p[t][rb:rb + 12, :])
                nc.sync.dma_start(aplane[12:24, c * CH:(c + 1) * CH],
                                  a1p[t][rb:rb + 12, :])

            # ---- idx spread (call-major sigma):
            # idxt[dk][16g+p, 32c+s] = i16p[12*(p//2)+dk, (p%2)*256+32c+s]
            #   => gather call c, output col j=s*16+p holds plane position
            #      l'(j) = 256*(j%16) + 32*c + j//16
            for dk in range(12):
                for g in range(8):
                    nc.sync.dma_start(idxt[dk][16 * g:16 * g + 8, :],
                                      i16p[0][dk:128:32, :])
                    nc.sync.dma_start(idxt[dk][16 * g + 8:16 * g + 16, :],
                                      i16p[1][dk:128:32, :])
            # ---- gather / broadcast / modulate (8 calls of 512) -----------
            shalf = [spool.tile([128, LH], dt.bfloat16, tag=f"s{kb}", name=f"s{kb}")
                     for kb in range(6)]
            ap_sig = aplane[:].rearrange("a (p c s) -> a c s p", p=16, c=8, s=32)
            for c in range(8):
                for dk in range(12):
                    gt = gpool.tile([128, CH], dt.bfloat16, tag=f"g{dk}", name=f"g{dk}")
                    at = apool.tile([128, CH], dt.bfloat16, tag=f"a{dk}", name=f"a{dk}")
                    nc.gpsimd.dma_gather(
                        gt[:].unsqueeze(1),
                        t_xT[dk // 3][:], idxt[dk][:, 32 * c:32 * c + 32],
                        num_idxs=CH, num_idxs_reg=CH, elem_size=128,
                        transpose=True, sbuf_tokens_per_rank=128,
                        sbuf_free_dim_per_rank=256)
                    bps = psb.tile([128, CH], dt.float32, tag="bcps", name="bcps")
                    nc.tensor.matmul(bps[:], t_wsel[dk][:], ap_sig[:, c],
                                     start=True, stop=True)
                    nc.scalar.copy(at[:], bps[:])
                    mt = mpool.tile([128, CH], dt.bfloat16, tag=f"mt{dk % 2}", name=f"mt{dk % 2}")
                    nc.vector.tensor_mul(mt[:], gt[:], at[:])
                    v1t = mpool.tile([64, CH], dt.bfloat16, tag=f"v1t{dk % 2}", name=f"v1t{dk % 2}")
                    nc.sync.dma_start(v1t[:], mt[64:128, :])
                    nc.vector.tensor_add(
                        shalf[dk // 2][64 * (dk % 2):64 * (dk % 2) + 64,
                                       c * CH:(c + 1) * CH],
                        mt[0:64, :], v1t[:])
            # ---- main matmuls ---------------------------------------------
            for cn in range(NCHUNK):
                for mb in range(2):
                    mps = psm.tile([128, CH], dt.float32, tag=f"mps{mb}", name=f"mps{mb}")
                    for kb in range(6):
                        rhs = shalf[kb][:].rearrange(
                            "a (c s p) -> a p c s", c=8, s=32, p=16)[:, 2 * cn:2 * cn + 2]
                        nc.tensor.matmul(mps[:], t_wmain[kb][mb][:], rhs,
                                         start=(kb == 0), stop=(kb == 5))
                    ot = opool.tile([128, CH], dt.float32, tag=f"ot{mb}", name=f"ot{mb}")
                    nc.scalar.activation(ot[:], mps[:], AF.Identity,
                                         bias=t_bmain[mb][:], scale=1.0)
                    nc.sync.dma_start(yout[mb, :, cn * CH:(cn + 1) * CH], ot[:])

    nc.compile()
    return nc


# ---------------------------------------------------------------------------

def _prep_core_inputs(x, w_off, b_off, w_mask, b_mask, weight, bias, b, h):
    q0 = h * LH - HALO
    xpad = np.zeros((C, W + 1), np.float32)
    lo, hi = max(0, q0), min(L, q0 + W + 1)
    xpad[:, lo - q0:hi - q0] = x[b][:, lo:hi]
    xpad_bf = xpad.astype(bf16)

    xT = np.zeros((4, 128, WROWS), bf16)
    for d in range(4):
        rows = np.concatenate([xpad_bf[d * 64:(d + 1) * 64, :W],
                               xpad_bf[d * 64:(d + 1) * 64, 1:W + 1]],
                              axis=0).T           # [W, 128] row e
        full = np.zeros((WROWS, 128), bf16)
        full[:W] = rows
        xT[d] = full.reshape(WROWS // 128, 128, 128).transpose(1, 0, 2) \
                    .reshape(128, WROWS)
    xP = np.ascontiguousarray(xpad_bf[:, :W].reshape(2, 128, W))

    wconv = np.zeros((2, 3, 128, 44), bf16)
    for cb in range(2):
        for k in range(3):
            wconv[cb, k, :, 0:12] = w_off[:, cb * 128:(cb + 1) * 128, k].T
            wconv[cb, k, :, 32:44] = w_mask[:, cb * 128:(cb + 1) * 128, k].T
    iotas = np.zeros((2, 128, CH), np.float32)
    col = np.arange(CH, dtype=np.float32)
    for t in range(2):
        for cb in range(4):
            for r in range(12):
                iotas[t, 32 * cb + r, :] = 512 * (4 * t + cb) + col + (r % 3) - 1 + HALO
    boff_c = b_off.astype(np.float32).reshape(12, 1)
    bmask_c = b_mask.astype(np.float32).reshape(12, 1)

    wmain = np.zeros((6, 2, 128, 128), bf16)
    for kb in range(6):
        for half in range(2):
            dk = 2 * kb + half
            d, k = dk // 3, dk % 3
            wblock = weight[:, d * 64:(d + 1) * 64, k]
            for mb in range(2):
                wmain[kb, mb, 64 * half:64 * half + 64, :] = \
                    wblock[mb * 128:(mb + 1) * 128, :].T
    bmain = bias.astype(np.float32).reshape(2, 128, 1)

    wsel = np.zeros((12, 24, 128), bf16)
    for dk in range(12):
        wsel[dk, dk, 0:64] = 1.0
        wsel[dk, 12 + dk, 64:128] = 1.0
    return {"xT": xT, "xP": xP, "wconv": wconv, "iotas": iotas,
            "boff": boff_c, "bmask": bmask_c,
            "wmain": wmain, "bmain": bmain, "wsel": wsel}


_CACHED = {}


def kernel(x, w_off, b_off, w_mask, b_mask, weight, bias):
    x = np.asarray(x, np.float32)
    w_off = np.asarray(w_off, np.float32)
    b_off = np.asarray(b_off, np.float32)
    w_mask = np.asarray(w_mask, np.float32)
    b_mask = np.asarray(b_mask, np.float32)
    weight = np.asarray(weight, np.float32)
    bias = np.asarray(bias, np.float32)

    if "nc" not in _CACHED:
        _CACHED["nc"] = build_program(1)
    nc = _CACHED["nc"]

    in_maps = [
        _prep_core_inputs(x, w_off, b_off, w_mask, b_mask, weight, bias,
                          core // 2, core % 2)
        for core in range(N_CORES)
    ]
    from concourse.bass_utils import run_bass_kernel_spmd
    res = run_bass_kernel_spmd(nc, in_maps, core_ids=list(range(N_CORES)))
    out = np.zeros((B, C, L), np.float32)
    for core in range(N_CORES):
        b, h = core // 2, core % 2
        y = res.results[core]["y"]
        out[b, 0:128, h * LH:(h + 1) * LH] = y[0]
        out[b, 128:256, h * LH:(h + 1) * LH] = y[1]
    return out


# revision 18
# speedup vs baseline: 1.0083x; 1.0083x over previous
"""Deformable Conv1d (B=4, C=256, L=8192, K=3, DG=4) on 8 Trainium2 cores.

Sharding: core = (sample b = core//2, L-half h = core%2); each core computes
out[b, :, h*4096:(h+1)*4096] from a haloed window of x[b].

Per-core pipeline:
  conv (PE, bf16): offset/mask convs as shifted-rhs matmuls + an iota matmul
    so PSUM holds p = off + l + (k-1) + HALO after the per-row drain bias.
  small chain (DVE, packed [96,512]): t = p mod 1, a0=(1-t)*m, a1=t*m,
    idx = int16(p - t) clamped.
  dma_gather (transpose=True) from per-dgroup transposed pair tables:
    row e = [x[c, e] for c in dgroup] ++ [x[c, e+1] for c]  (256B bf16 rows).
  A-broadcast (PE selector matmul): replicates a0 to partitions 0-63 and a1
    to 64-127, reading the a-plane through a sigma_c-permuted rhs AP.
  modulate (DVE): M = G * A;  S[64*(dk%2)+...] = M[0:64] + M[64:128].
  main matmul (PE): out = W2 @ S + bias with sigma_c-unpermuting rhs AP.
"""
import sys
sys.path.insert(0, '/opt/trn_rl_repo')
from contextlib import ExitStack
import numpy as np
import ml_dtypes

import concourse.bass as bass
import concourse.tile as tile
from concourse import bacc, mybir

dt = mybir.dt
bf16 = ml_dtypes.bfloat16

B, C, L = 4, 256, 8192
N_CORES = 8
LH = L // 2
HALO = 17
W = LH + 2 * HALO          # 4130 window positions
WROWS = 33 * 128           # 4224 padded rows in pair tables
NCHUNK = 8
CH = 512
NBATCH = 4                 # 2-chunk modulate batches
BCH = 2 * CH
AF = mybir.ActivationFunctionType
ALU = mybir.AluOpType


def build_program(n_reps=1):
    nc = bacc.Bacc("TRN2", target_bir_lowering=False, debug=False,
                   enable_asserts=True, num_devices=N_CORES)

    def din(name, shape, dty):
        return nc.dram_tensor(name, shape, dty, kind="ExternalInput").ap()

    xT = din("xT", (4, 128, WROWS), dt.bfloat16)
    xP = din("xP", (2, 128, W), dt.bfloat16)
    wconv = din("wconv", (2, 3, 128, 44), dt.bfloat16)
    iotas = din("iotas", (2, 128, CH), dt.float32)
    boff = din("boff", (12, 1), dt.float32)
    bmask = din("bmask", (12, 1), dt.float32)
    wmain = din("wmain", (6, 2, 128, 128), dt.bfloat16)
    bmain = din("bmain", (2, 128, 1), dt.float32)
    wsel = din("wsel", (12, 24, 128), dt.bfloat16)
    yout = nc.dram_tensor("y", (2, 128, LH), dt.float32, kind="ExternalOutput").ap()

    with ExitStack() as ctx:
        tc = ctx.enter_context(tile.TileContext(nc))
        cpool = ctx.enter_context(tc.tile_pool(name="const", bufs=1))
        ppool = ctx.enter_context(tc.tile_pool(name="planes", bufs=1))
        gpool = ctx.enter_context(tc.tile_pool(name="g", bufs=2))
        mpool = ctx.enter_context(tc.tile_pool(name="mtp", bufs=2))
        apool = ctx.enter_context(tc.tile_pool(name="a", bufs=1))
        spool = ctx.enter_context(tc.tile_pool(name="s", bufs=1))
        opool = ctx.enter_context(tc.tile_pool(name="o", bufs=2))
        psc = ctx.enter_context(tc.tile_pool(name="psc", bufs=2, space="PSUM"))
        psb = ctx.enter_context(tc.tile_pool(name="psb", bufs=2, space="PSUM"))
        psm = ctx.enter_context(tc.tile_pool(name="psm", bufs=2, space="PSUM"))

        t_xT = [cpool.tile([128, WROWS], dt.bfloat16, tag=f"xT{d}", name=f"xT{d}") for d in range(4)]
        for d in range(4):
            nc.sync.dma_start(t_xT[d][:], xT[d])
        t_xP = [cpool.tile([128, W], dt.bfloat16, tag=f"xP{cb}", name=f"xP{cb}") for cb in range(2)]
        for cb in range(2):
            nc.sync.dma_start(t_xP[cb][:], xP[cb])
        t_wconv = [[cpool.tile([128, 44], dt.bfloat16, tag=f"wc{cb}{k}", name=f"wc{cb}{k}")
                    for k in range(3)] for cb in range(2)]
        for cb in range(2):
            for k in range(3):
                nc.sync.dma_start(t_wconv[cb][k][:], wconv[cb, k])
        t_iot = [cpool.tile([128, CH], dt.float32, tag=f"iot{t}", name=f"iot{t}")
                 for t in range(2)]
        for t in range(2):
            nc.sync.dma_start(t_iot[t][:], iotas[t])
        t_boff = cpool.tile([12, 1], dt.float32, name="boff")
        nc.sync.dma_start(t_boff[:], boff[:])
        t_bmask = cpool.tile([12, 1], dt.float32, name="bmask")
        nc.sync.dma_start(t_bmask[:], bmask[:])
        t_wmain = [[cpool.tile([128, 128], dt.bfloat16, tag=f"wm{kb}{mb}", name=f"wm{kb}{mb}")
                    for mb in range(2)] for kb in range(6)]
        for kb in range(6):
            for mb in range(2):
                nc.sync.dma_start(t_wmain[kb][mb][:], wmain[kb, mb])
        t_bmain = [cpool.tile([128, 1], dt.float32, tag=f"bm{mb}", name=f"bm{mb}") for mb in range(2)]
        for mb in range(2):
            nc.sync.dma_start(t_bmain[mb][:], bmain[mb])
        t_wsel = [cpool.tile([24, 128], dt.bfloat16, tag=f"sel{dk}", name=f"sel{dk}") for dk in range(12)]
        for dk in range(12):
            nc.sync.dma_start(t_wsel[dk][:], wsel[dk])

        for _rep in range(n_reps):
            pk = [ppool.tile([128, CH], dt.float32, tag=f"pk{t}", name=f"pk{t}") for t in range(2)]
            mk = [ppool.tile([128, CH], dt.float32, tag=f"mk{t}", name=f"mk{t}") for t in range(2)]
            ttl = [ppool.tile([128, CH], dt.float32, tag=f"ttl{t}", name=f"ttl{t}") for t in range(2)]
            p0f = [ppool.tile([128, CH], dt.float32, tag=f"p0f{t}", name=f"p0f{t}") for t in range(2)]
            a0p = [ppool.tile([128, CH], dt.bfloat16, tag=f"a0p{t}", name=f"a0p{t}") for t in range(2)]
            a1p = [ppool.tile([128, CH], dt.bfloat16, tag=f"a1p{t}", name=f"a1p{t}") for t in range(2)]
            i16p = [ppool.tile([128, CH], dt.int16, tag=f"i16p{t}", name=f"i16p{t}") for t in range(2)]
            aplane = ppool.tile([24, LH], dt.bfloat16, tag="aplane", name="aplane")
            idxt = [ppool.tile([128, LH // 16], dt.int16, tag=f"idx{dk}", name=f"idx{dk}")
                    for dk in range(12)]

            # ---- conv + drains --------------------------------------------
            for c in range(NCHUNK):
                ps = psc.tile([64, CH], dt.float32, tag="convps", name="convps")
                for cb in range(2):
                    for k in range(3):
                        rhs = t_xP[cb][:, c * CH + HALO - 1 + k:
                                       c * CH + HALO - 1 + k + CH]
                        nc.tensor.matmul(ps[0:44, :], t_wconv[cb][k][:], rhs,
                                         start=(cb == 0 and k == 0),
                                         stop=(cb == 1 and k == 2))
                t, rb = c // 4, 32 * (c % 4)
                nc.scalar.activation(pk[t][rb:rb + 12, :], ps[0:12, :], AF.Identity,
                                     bias=t_boff[:], scale=1.0)
                nc.scalar.activation(mk[t][rb:rb + 12, :], ps[32:44, :], AF.Sigmoid,
                                     bias=t_bmask[:], scale=1.0)

            # ---- small chain (packed [96, CH]) ----------------------------
            for t in range(2):
                nc.vector.tensor_add(pk[t][:], pk[t][:], t_iot[t][:])
                nc.vector.tensor_copy(i16p[t][:], pk[t][:])         # ~round/trunc
                nc.vector.tensor_copy(p0f[t][:], i16p[t][:])
                nc.vector.tensor_sub(ttl[t][:], pk[t][:], p0f[t][:])  # d = p - i
                # floor fixup: if d < 0: i -= 1, d += 1
                nc.vector.tensor_scalar(pk[t][:], ttl[t][:], 0.0, None, ALU.is_lt)
                nc.vector.tensor_sub(p0f[t][:], p0f[t][:], pk[t][:])
                nc.vector.tensor_add(ttl[t][:], ttl[t][:], pk[t][:])  # t in [0,1)
                nc.vector.tensor_mul(ttl[t][:], ttl[t][:], mk[t][:])
                nc.vector.tensor_copy(a1p[t][:], ttl[t][:])
                nc.vector.tensor_sub(a0p[t][:], mk[t][:], ttl[t][:])
                nc.vector.tensor_copy(i16p[t][:], p0f[t][:])
                nc.vector.tensor_scalar_max(i16p[t][:], i16p[t][:], 0)
                nc.vector.tensor_scalar_min(i16p[t][:], i16p[t][:], W - 1)

            # ---- unpack to planes -----------------------------------------
            for c in range(NCHUNK):
                t, rb = c // 4, 32 * (c % 4)
                nc.sync.dma_start(aplane[0:12, c * CH:(c + 1) * CH],
                                  a0p[t][rb:rb + 12, :])
                nc.sync.dma_start(aplane[12:24, c * CH:(c + 1) * CH],
                                  a1p[t][rb:rb + 12, :])

            # ---- idx spread (call-major sigma):
            # idxt[dk][16g+p, 32c+s] = i16p[12*(p//2)+dk, (p%2)*256+32c+s]
            #   => gather call c, output col j=s*16+p holds plane position
            #      l'(j) = 256*(j%16) + 32*c + j//16
            for dk in range(12):
                for g in range(8):
                    nc.sync.dma_start(idxt[dk][16 * g:16 * g + 8, :],
                                      i16p[0][dk:128:32, :])
                    nc.sync.dma_start(idxt[dk][16 * g + 8:16 * g + 16, :],
                                      i16p[1][dk:128:32, :])
            # ---- gather / broadcast / modulate (8 calls of 512) -----------
            shalf = [spool.tile([128, LH], dt.bfloat16, tag=f"s{kb}", name=f"s{kb}")
                     for kb in range(6)]
            ap_sig = aplane[:].rearrange("a (p c s) -> a c s p", p=16, c=8, s=32)
            for c in range(8):
                for dk in range(12):
                    gt = gpool.tile([128, CH], dt.bfloat16, tag=f"g{dk}", name=f"g{dk}")
                    at = apool.tile([128, CH], dt.bfloat16, tag=f"a{dk}", name=f"a{dk}")
                    nc.gpsimd.dma_gather(
                        gt[:].unsqueeze(1),
                        t_xT[dk // 3][:], idxt[dk][:, 32 * c:32 * c + 32],
                        num_idxs=CH, num_idxs_reg=CH, elem_size=128,
                        transpose=True, sbuf_tokens_per_rank=128,
                        sbuf_free_dim_per_rank=256)
                    bps = psb.tile([128, CH], dt.float32, tag="bcps", name="bcps")
                    nc.tensor.matmul(bps[:], t_wsel[dk][:], ap_sig[:, c],
                                     start=True, stop=True)
                    nc.scalar.copy(at[:], bps[:])
                    mt = mpool.tile([128, CH], dt.bfloat16, tag=f"mt{dk % 2}", name=f"mt{dk % 2}")
                    nc.vector.tensor_mul(mt[:], gt[:], at[:])
                    v1t = mpool.tile([64, CH], dt.bfloat16, tag=f"v1t{dk % 2}", name=f"v1t{dk % 2}")
                    nc.sync.dma_start(v1t[:], mt[64:128, :])
                    nc.vector.tensor_add(
                        shalf[dk // 2][64 * (dk % 2):64 * (dk % 2) + 64,
                                       c * CH:(c + 1) * CH],
                        mt[0:64, :], v1t[:])
            # ---- main matmuls ---------------------------------------------
            for cn in range(NCHUNK):
                for mb in range(2):
                    mps = psm.tile([128, CH], dt.float32, tag=f"mps{mb}", name=f"mps{mb}")
                    for kb in range(6):
                        rhs = shalf[kb][:].rearrange(
                            "a (c s p) -> a p c s", c=8, s=32, p=16)[:, 2 * cn:2 * cn + 2]
                        nc.tensor.matmul(mps[:], t_wmain[kb][mb][:], rhs,
                                         start=(kb == 0), stop=(kb == 5))
                    ot = opool.tile([128, CH], dt.float32, tag=f"ot{mb}", name=f"ot{mb}")
                    nc.scalar.activation(ot[:], mps[:], AF.Identity,
                                         bias=t_bmain[mb][:], scale=1.0)
                    nc.sync.dma_start(yout[mb, :, cn * CH:(cn + 1) * CH], ot[:])

    nc.compile()
    return nc


# ---------------------------------------------------------------------------

def _prep_core_inputs(x, w_off, b_off, w_mask, b_mask, weight, bias, b, h):
    q0 = h * LH - HALO
    xpad = np.zeros((C, W + 1), np.float32)
    lo, hi = max(0, q0), min(L, q0 + W + 1)
    xpad[:, lo - q0:hi - q0] = x[b][:, lo:hi]
    xpad_bf = xpad.astype(bf16)

    xT = np.zeros((4, 128, WROWS), bf16)
    for d in range(4):
        rows = np.concatenate([xpad_bf[d * 64:(d + 1) * 64, :W],
                               xpad_bf[d * 64:(d + 1) * 64, 1:W + 1]],
                              axis=0).T           # [W, 128] row e
        full = np.zeros((WROWS, 128), bf16)
        full[:W] = rows
        xT[d] = full.reshape(WROWS // 128, 128, 128).transpose(1, 0, 2) \
                    .reshape(128, WROWS)
    xP = np.ascontiguousarray(xpad_bf[:, :W].reshape(2, 128, W))

    wconv = np.zeros((2, 3, 128, 44), bf16)
    for cb in range(2):
        for k in range(3):
            wconv[cb, k, :, 0:12] = w_off[:, cb * 128:(cb + 1) * 128, k].T
            wconv[cb, k, :, 32:44] = w_mask[:, cb * 128:(cb + 1) * 128, k].T
    iotas = np.zeros((2, 128, CH), np.float32)
    col = np.arange(CH, dtype=np.float32)
    for t in range(2):
        for cb in range(4):
            for r in range(12):
                iotas[t, 32 * cb + r, :] = 512 * (4 * t + cb) + col + (r % 3) - 1 + HALO
    boff_c = b_off.astype(np.float32).reshape(12, 1)
    bmask_c = b_mask.astype(np.float32).reshape(12, 1)

    wmain = np.zeros((6, 2, 128, 128), bf16)
    for kb in range(6):
        for half in range(2):
            dk = 2 * kb + half
            d, k = dk // 3, dk % 3
            wblock = weight[:, d * 64:(d + 1) * 64, k]
            for mb in range(2):
                wmain[kb, mb, 64 * half:64 * half + 64, :] = \
                    wblock[mb * 128:(mb + 1) * 128, :].T
    bmain = bias.astype(np.float32).reshape(2, 128, 1)

    wsel = np.zeros((12, 24, 128), bf16)
    for dk in range(12):
        wsel[dk, dk, 0:64] = 1.0
        wsel[dk, 12 + dk, 64:128] = 1.0
    return {"xT": xT, "xP": xP, "wconv": wconv, "iotas": iotas,
            "boff": boff_c, "bmask": bmask_c,
            "wmain": wmain, "bmain": bmain, "wsel": wsel}


_CACHED = {}


def kernel(x, w_off, b_off, w_mask, b_mask, weight, bias):
    x = np.asarray(x, np.float32)
    w_off = np.asarray(w_off, np.float32)
    b_off = np.asarray(b_off, np.float32)
    w_mask = np.asarray(w_mask, np.float32)
    b_mask = np.asarray(b_mask, np.float32)
    weight = np.asarray(weight, np.float32)
    bias = np.asarray(bias, np.float32)

    if "nc" not in _CACHED:
        _CACHED["nc"] = build_program(1)
    nc = _CACHED["nc"]

    in_maps = [
        _prep_core_inputs(x, w_off, b_off, w_mask, b_mask, weight, bias,
                          core // 2, core % 2)
        for core in range(N_CORES)
    ]
    from concourse.bass_utils import run_bass_kernel_spmd
    res = run_bass_kernel_spmd(nc, in_maps, core_ids=list(range(N_CORES)))
    out = np.zeros((B, C, L), np.float32)
    for core in range(N_CORES):
        b, h = core // 2, core % 2
        y = res.results[core]["y"]
        out[b, 0:128, h * LH:(h + 1) * LH] = y[0]
        out[b, 128:256, h * LH:(h + 1) * LH] = y[1]
    return out
